# revision 20
# baseline (speedup 1.0000x reference)
"""APT encoder scatter kernel for TRN2 (8 NeuronCores, data-parallel over batch).

Problem: scatter patch tokens [B, P*BS, D] to a dense grid [B, H, W, T, BS, D]
per positions [B, P, 4] (rows y, x, size, t), broadcasting size-2 patches over
their 2x2 cell footprint.

Design (per core, one sample). Out row for cell (y, x, t) is 128*y + 4*x + t;
each row is BS*D = 2304 f32 = 9216 B. Every patch writes its base cell
(slot 0, always valid). The 2x2 broadcast of coarse (size==2) patches issues
zero wasted DMA descriptors via an on-device compaction computed entirely on
the compute engines (no DRAM round trip):

  1. base = 128*y + 4*x + t and is2 = (size >= 2), elementwise on positions.
  2. rank = exclusive prefix sum of is2 in patch order (free-dim scan +
     cross-partition exclusive prefix via a strictly-triangular matmul).
  3. Compacted (pid, base, hit) table via one-hot selection matmuls:
     Sel[q, r] = (rank[q] == r) * is2[q] accumulated over 20 patch-column
     chunks into PSUM, for 4 chunks of 128 compact slots each.
  4. Indirect-gather the coarse token rows by compacted pid (4 tiles of
     128), then 3 all-valid scatters per tile at base + {4, 128, 132}
     (cells (y, x+1), (y+1, x), (y+1, x+1)). Unused compact slots (hit=0)
     target a dummy out row past the real output.

DMA strategy: every bulk transfer (token loads, gathers, all scatters) goes
through the ONE SWDGE queue, so the SDMA engines stream a single ring with no
per-packet queue switching (measured ~3x per-descriptor overhead when two
queues interleave). Token loads run 12 tiles ahead of their scatters; the
compaction runs on DVE/PE concurrently with the token stream. Only
provably-false WAW edges (scatters to disjoint rows of the same tensor) are
demoted to issue-order edges; all real RAW/WAR semaphores stay.

Per-core HBM traffic: reads 23.6 (tokens) + 4.7 (coarse re-read) MB, writes
37.7 MB — no wasted bytes.
"""

import numpy as np

import concourse.bass as bass
import concourse.bacc as bacc
import concourse.mybir as mybir
import concourse.tile as tile
from concourse.instruction_name_ordered_set import InstructionNameOrderedSet
from concourse.bass_utils import run_bass_kernel_spmd

B = 8
H, W, T, BS, D = 32, 32, 4, 3, 768
P = 2560             # patches per sample
ROW = BS * D         # 2304 f32 per patch/cell row
NCELL = H * W * T    # 4096 output rows per sample
PPART = 20           # patches per SBUF partition (2560 = 128 * 20)
NT = PPART           # scatter tiles per sample
NC2 = P // 5         # compact capacity for coarse patches (512 = exact count)
NG = NC2 // 128      # coarse gather tiles (4)
DUMMY = NCELL        # out row absorbing writes from unused compact slots
OUTROWS = 4352       # NCELL + pad >= DUMMY + 132 so slop writes stay inside
PREFETCH = 8        # token tiles in flight ahead of their scatters

_CACHE = {}


def _build():
    nc = bacc.Bacc("TRN2", target_bir_lowering=False, debug=False, num_devices=B)
    tok = nc.declare_dram_parameter("tok", [P, ROW], mybir.dt.float32, isOutput=False)
    pos = nc.declare_dram_parameter("pos", [P, 4], mybir.dt.int32, isOutput=False)
    out = nc.declare_dram_parameter(
        "out", [OUTROWS, ROW], mybir.dt.float32, isOutput=True
    )

    i32 = mybir.dt.int32
    f32 = mybir.dt.float32
    Op = mybir.AluOpType

    with tile.TileContext(nc) as tc:
        with (
            tc.tile_pool(name="meta", bufs=1) as meta,
            tc.tile_pool(name="sel", bufs=4) as selp,
            tc.tile_pool(name="toks", bufs=PREFETCH) as toks,
            tc.tile_pool(name="coarse", bufs=NG) as cpool,
            tc.tile_pool(name="psum", bufs=1, space="PSUM") as psum,
        ):
            # ---- positions: partition p holds patches [20p, 20p+19] ----
            pos_sb = meta.tile([128, PPART * 4], i32)
            nc.sync.dma_start(
                out=pos_sb[:], in_=pos[:].rearrange("(p i) c -> p (i c)", p=128)
            )
            pos3 = pos_sb[:].rearrange("p (i c) -> p i c", c=4)
            y = pos3[:, :, 0]
            x = pos3[:, :, 1]
            s = pos3[:, :, 2]
            t = pos3[:, :, 3]

            # ---- start the token stream immediately: prefetch tiles ----
            tok_r = tok[:].rearrange("(p i) r -> p i r", i=PPART)
            tok_tiles = []
            for i in range(min(PREFETCH, NT)):
                tok_t = toks.tile([128, ROW], f32)
                nc.gpsimd.dma_start(out=tok_t[:], in_=tok_r[:, i, :])
                tok_tiles.append(tok_t)

            # ---- base cell row and coarse flag ----
            base = meta.tile([128, PPART], i32)
            is2 = meta.tile([128, PPART], i32)
            nc.vector.tensor_scalar(
                out=base[:], in0=y, scalar1=128, scalar2=None, op0=Op.mult
            )
            nc.vector.scalar_tensor_tensor(
                out=base[:], in0=x, scalar=4, in1=base[:], op0=Op.mult, op1=Op.add
            )
            nc.vector.tensor_tensor(out=base[:], in0=base[:], in1=t, op=Op.add)
            nc.vector.tensor_scalar(
                out=is2[:], in0=s, scalar1=2, scalar2=None, op0=Op.is_ge
            )

            # ---- rank: exclusive prefix sum of is2 in patch order ----
            zeros = meta.tile([128, PPART], i32)
            nc.vector.memset(zeros[:], 0)
            scan = meta.tile([128, PPART], i32)
            nc.vector.tensor_tensor_scan(
                out=scan[:],
                data0=is2[:],
                data1=zeros[:],
                initial=0.0,
                op0=Op.add,
                op1=Op.add,
            )
            tot_f = meta.tile([128, 1], f32)
            nc.vector.tensor_copy(tot_f[:], scan[:, PPART - 1 : PPART])
            ones = meta.tile([128, 128], f32)
            nc.vector.memset(ones[:], 1.0)
            # M[k, p] = 1 iff p > k  ->  (lhsT.T @ tot)[p] = sum_{k<p} tot[k]
            trimat = meta.tile([128, 128], f32)
            nc.gpsimd.affine_select(
                out=trimat[:],
                in_=ones[:],
                pattern=[[1, 128]],
                compare_op=Op.is_gt,
                fill=0.0,
                base=0,
                channel_multiplier=-1,
            )
            part_off_ps = psum.tile([128, 1], f32)
            nc.tensor.matmul(
                out=part_off_ps[:], lhsT=trimat[:], rhs=tot_f[:], start=True, stop=True
            )
            part_off = meta.tile([128, 1], f32)
            nc.vector.tensor_copy(part_off[:], part_off_ps[:])

            # rank as f32 (used as a per-partition scalar against iota rows)
            rank_f = meta.tile([128, PPART], f32)
            nc.vector.tensor_tensor(
                out=rank_f[:], in0=scan[:], in1=is2[:], op=Op.subtract
            )
            nc.vector.tensor_scalar(
                out=rank_f[:], in0=rank_f[:], scalar1=part_off[:, 0:1], scalar2=None,
                op0=Op.add,
            )
            is2_f = meta.tile([128, PPART], f32)
            nc.vector.tensor_copy(is2_f[:], is2[:])

            # ---- rhs for the compaction matmuls: (pid, base, 1) as f32 ----
            pid = meta.tile([128, PPART], i32)
            nc.gpsimd.iota(
                out=pid[:], pattern=[[1, PPART]], base=0, channel_multiplier=PPART
            )
            rhs_f = meta.tile([128, PPART * 3], f32)
            rhs3 = rhs_f[:].rearrange("p (i c) -> p i c", c=3)
            nc.vector.tensor_copy(rhs3[:, :, 0], pid[:])
            nc.vector.tensor_copy(rhs3[:, :, 1], base[:])
            nc.vector.memset(rhs3[:, :, 2], 1.0)

            # iota row 0..511 replicated on every partition, as f32
            iota_i = meta.tile([128, NC2], i32)
            nc.gpsimd.iota(
                out=iota_i[:], pattern=[[1, NC2]], base=0, channel_multiplier=0
            )
            iota_f = meta.tile([128, NC2], f32)
            nc.vector.tensor_copy(iota_f[:], iota_i[:])

            # ---- compacted (pid, base, hit) per 128-slot chunk ----
            gidxs = []   # [128, 1] i32 gather indices per chunk
            offcs = []   # [128, 3] i32 scatter offsets per chunk
            for g in range(NG):
                acc = psum.tile([128, 3], f32, tag=f"acc{g}", space="PSUM")
                for i in range(PPART):
                    sel = selp.tile([128, 128], f32)
                    # Sel[p, r] = (iota[r] == rank[p,i]) * is2[p,i]
                    nc.vector.tensor_scalar(
                        out=sel[:],
                        in0=iota_f[:, g * 128 : (g + 1) * 128],
                        scalar1=rank_f[:, i : i + 1],
                        scalar2=is2_f[:, i : i + 1],
                        op0=Op.is_equal,
                        op1=Op.mult,
                    )
                    nc.tensor.matmul(
                        out=acc[:],
                        lhsT=sel[:],
                        rhs=rhs3[:, i, :],
                        start=(i == 0),
                        stop=(i == PPART - 1),
                    )
                cpb = meta.tile([128, 3], f32, tag=f"cpb{g}")
                nc.vector.tensor_copy(cpb[:], acc[:])
                gidx = meta.tile([128, 1], i32, tag=f"gidx{g}")
                nc.vector.tensor_copy(gidx[:], cpb[:, 0:1])
                gidxs.append(gidx)
                # gbase = base + (1 - hit) * DUMMY  (unused slots -> dummy row)
                gbase = meta.tile([128, 1], f32, tag=f"gbase{g}")
                nc.vector.scalar_tensor_tensor(
                    out=gbase[:],
                    in0=cpb[:, 2:3],
                    scalar=-DUMMY,
                    in1=cpb[:, 1:2],
                    op0=Op.mult,
                    op1=Op.add,
                )
                nc.vector.tensor_scalar(
                    out=gbase[:], in0=gbase[:], scalar1=DUMMY, scalar2=None, op0=Op.add
                )
                offc = meta.tile([128, 3], i32, tag=f"offc{g}")
                for jj, cj in enumerate((4, 128, 132)):
                    nc.vector.tensor_scalar(
                        out=offc[:, jj : jj + 1],
                        in0=gbase[:],
                        scalar1=cj,
                        scalar2=None,
                        op0=Op.add,
                    )
                offcs.append(offc)

            # ---- steady state: scatter tile i, prefetch tile i+PREFETCH;
            # gathers and coarse scatters woven into the stream ----
            out_scats = []
            coarse_tiles = {}
            # gather g is emitted after slot-0 scatter (2 + 2g); its 3 coarse
            # scatters follow ~4 tiles later
            gather_at = {2 + 2 * g: g for g in range(NG)}
            cscat_at = {}
            for g in range(NG):
                for jj in range(3):
                    cscat_at.setdefault(8 + 3 * g + jj, []).append((g, jj))
            for i in range(NT):
                sinst = nc.gpsimd.indirect_dma_start(
                    out=out[:],
                    out_offset=bass.IndirectOffsetOnAxis(ap=base[:, i : i + 1], axis=0),
                    in_=tok_tiles[i][:],
                    in_offset=None,
                )
                out_scats.append(sinst)
                if i + PREFETCH < NT:
                    tok_t = toks.tile([128, ROW], f32)
                    nc.gpsimd.dma_start(out=tok_t[:], in_=tok_r[:, i + PREFETCH, :])
                    tok_tiles.append(tok_t)
                if i in gather_at:
                    g = gather_at[i]
                    coarse_t = cpool.tile([128, ROW], f32)
                    nc.gpsimd.indirect_dma_start(
                        out=coarse_t[:],
                        out_offset=None,
                        in_=tok[:],
                        in_offset=bass.IndirectOffsetOnAxis(ap=gidxs[g][:], axis=0),
                    )
                    coarse_tiles[g] = coarse_t
                for g, jj in cscat_at.get(i, []):
                    cinst = nc.gpsimd.indirect_dma_start(
                        out=out[:],
                        out_offset=bass.IndirectOffsetOnAxis(
                            ap=offcs[g][:, jj : jj + 1], axis=0
                        ),
                        in_=coarse_tiles[g][:],
                        in_offset=None,
                    )
                    out_scats.append(cinst)

            # ---- demote provably-false WAW edges (disjoint-row scatters
            # to the same tensor); all real RAW/WAR semaphores stay ----
            names = {d.ins.name for d in out_scats}
            for dinst in out_scats:
                ins = dinst.ins
                sync_deps = list(ins.sync_dependency_names())
                demote = [n for n in sync_deps if n in names]
                if demote:
                    ins.set_sync_dependencies(
                        InstructionNameOrderedSet(
                            [n for n in sync_deps if n not in demote]
                        )
                    )
                    ins.set_nosync_dependencies(
                        InstructionNameOrderedSet(
                            list(ins.nosync_dependency_names()) + demote
                        )
                    )

    nc.compile()
    return nc


def _run(modality_tokens, positions, trace=False, tmpdir=None):
    nc = _CACHE.get("nc")
    if nc is None:
        nc = _CACHE["nc"] = _build()
    toks = np.ascontiguousarray(np.asarray(modality_tokens, dtype=np.float32)).reshape(
        B, P, ROW
    )
    poss = np.ascontiguousarray(np.asarray(positions, dtype=np.int32))
    in_maps = [{"tok": toks[b], "pos": poss[b]} for b in range(B)]
    res = run_bass_kernel_spmd(
        nc, in_maps, core_ids=list(range(B)), trace=trace, tmpdir=tmpdir
    )
    outf = np.stack([res.results[b]["out"][:NCELL] for b in range(B)])
    return outf.reshape(B, H, W, T, BS, D), res


def kernel(modality_tokens, positions):
    outf, _ = _run(modality_tokens, positions)
    return outf
